# revision 15
# baseline (speedup 1.0000x reference)
"""Causal self-attention with RoPE on 8 Trainium2 NeuronCores.

Problem: B=4, S=4096, E=64, H=4 heads x D=16, fp32 in/out.

Sharding: core c handles batch b = c//2 and head-pair hp = c%2 (heads 2*hp,
2*hp+1).  Every core runs the IDENTICAL program (SPMD) -- per-core behavior
comes only from the data (x[b] and per-head weight slices).  Each core
returns the partial output projection sum over its two heads; the host adds
the two partials per batch.

Device algorithm (per core, per head):
  - x^T [64,S] DMA'd pre-transposed (host does the transpose)
  - K^T,Q^T projections as lhsT.T@x^T (scale 1/sqrt(D) folded into Wq);
    RoPE applied as  rot = proj * cos + proj_shuf * sin  where proj_shuf
    comes from a sign/permuted weight matrix (R@W) -- no cross-partition ops
  - scores computed TRANSPOSED: S^T[k',q] over d=16 contraction, with the
    4 (kt-parity x head) score matmuls on the four 32-row PE row tiles
  - softmax exp: ScalarE ACTIVATE per kt-tile (a VectorE Schraudolph exp2
    fallback -- tensor_scalar mult+add writing int16 bits that ARE the
    bf16 pattern of 2^(s*log2e) -- is selectable via KEXP_MOD but measured
    slower: the DVE is the secondary pacer)
  - causal mask applied post-exp with affine_select (fill 0) on GpSimd
  - attended^T accumulated on the two 64-partition PE column tiles (head
    -> psum half), V augmented with a ones column so row 64hh+32
    accumulates the softmax denominator; attended trails the scores by two
    kt-pairs so row<->column PE mode switches halve
  - normalize with reciprocal + per-chunk scalar multiplies, PE-transposed
    denominators, output DMA'd row-major; projection/normalize/V work is
    spread across kt-pair steps so the exp engine never starves
"""

import sys

sys.path.insert(0, "/opt/trn_rl_repo")

import numpy as np
import ml_dtypes

B, S, E, H, D = 4, 4096, 64, 4, 16
NCORES = 8
NKT = S // 128  # 32 k-tiles of 128
NQC = S // 512  # 8 q-chunks of 512

BF16 = ml_dtypes.bfloat16

# Schraudolph fast-exp on bf16 bits: bits = rint(s*EXPA + EXPB); the int16
# pattern read as bf16 is exp(s)*(1 +- 3.3%), zero-ish mean after softmax
# normalization.  Validated bit-exact vs numpy rint on HW.
LOG2E = 1.4426950408889634
EXPA = 128.0 * LOG2E
EXPC = 5.60
EXPB = 127.0 * 128.0 - EXPC
# fraction of kt-tiles whose exp runs on the DVE instead of ScalarE:
# tile i -> DVE iff i % EXP_MOD == EXP_PHASE.  Default: ALL tiles on
# ScalarE -- measured faster than any DVE offload split, because the DVE
# (rope multiplies + casts + normalize tail) is the secondary pacer and
# exp tiles in its queue delay the score->attended chain, while ScalarE
# has slack.  The Schraudolph path stays available via KEXP_MOD.
EXP_MOD = int(__import__('os').environ.get('KEXP_MOD', 1 << 30))
EXP_PHASE = 2

_CACHE: dict = {}


def _rope_tables():
    # cos/sin[16*hh + d, s] = cos/sin(s * invfreq[d//2]); same for both heads
    pos = np.arange(S, dtype=np.float64)
    pair = np.arange(0, D, 2, dtype=np.float64)  # 0,2,..,14
    inv = 1.0 / (10000.0 ** (pair / D))          # [8]
    ang = pos[None, :] * inv[:, None]            # [8, S]
    cos8, sin8 = np.cos(ang), np.sin(ang)
    cos16 = np.repeat(cos8, 2, axis=0)           # [16, S] rows 2p,2p+1 equal
    sin16 = np.repeat(sin8, 2, axis=0)
    z16 = np.zeros_like(cos16)
    # [112, S]: rows 0-15, 32-47, 64-79, 96-111 hold cos16 (both heads, both
    # kt-parity copies); gap rows zero.
    cos112 = np.concatenate([cos16, z16, cos16, z16, cos16, z16, cos16], axis=0)
    sin112 = np.concatenate([sin16, z16, sin16, z16, sin16, z16, sin16], axis=0)
    return cos112.astype(BF16), sin112.astype(BF16)


def _shuffle_rows(w):
    # (R w)[2p] = -w[2p+1], (R w)[2p+1] = w[2p]   (rope partner)
    ws = np.empty_like(w)
    ws[0::2] = -w[1::2]
    ws[1::2] = w[0::2]
    return ws


def make_core_inputs(x, Wq, Wk, Wv, Wo, core):
    """Build the per-core input map (all host-side numpy)."""
    b, hp = core // 2, core % 2
    rs = slice(32 * hp, 32 * hp + 32)  # rows of the 2 heads in W{q,k,v}
    scale = 1.0 / np.sqrt(np.float32(D))

    wq_sel = (Wq[rs] * scale).astype(np.float32)  # [32, 64]
    wk_sel = Wk[rs].astype(np.float32)
    cos112, sin112 = _CACHE.setdefault("rope", _rope_tables())

    def gap112(w32):
        # [32,64] head rows -> [64,112] lhsT with head hh at cols
        # {32*hh, 64+32*hh} + 0:16 (duplicated for the kt-parity row groups)
        out = np.zeros((64, 112), np.float32)
        out[:, 0:16] = w32[0:16].T
        out[:, 32:48] = w32[16:32].T
        out[:, 64:80] = w32[0:16].T
        out[:, 96:112] = w32[16:32].T
        return out

    # plain and rope-shuffled projection weights stacked [128, 112]: rows
    # 0:64 feed PE row tile (0,0), rows 64:128 feed tile (64,0) so the two
    # projection matmuls stream concurrently
    wqc = np.concatenate([gap112(wq_sel), gap112(_shuffle_rows(wq_sel))], 0)
    wkc = np.concatenate([gap112(wk_sel), gap112(_shuffle_rows(wk_sel))], 0)
    return {
        "xt": np.ascontiguousarray(x[b].T).astype(BF16),              # [64,S]
        "wqc": np.ascontiguousarray(wqc).astype(BF16),                # [128,112]
        "wkc": np.ascontiguousarray(wkc).astype(BF16),
        "wv": np.ascontiguousarray(Wv[rs].T).astype(BF16),            # [64,32]
        # wo[d, hh, e] = Wo[e, 16*(2hp+hh)+d]
        "wo": np.ascontiguousarray(
            Wo[:, rs].reshape(E, 2, D).transpose(2, 1, 0)
        ).astype(BF16),                                               # [16,2,64]
        "cost": cos112,
        "sint": sin112,
        "idt": np.eye(128, dtype=BF16),
    }


def partial_reference(inp):
    """Numpy reference of ONE core's partial output (for testing)."""
    x = inp["xt"].astype(np.float64).T
    cos = inp["cost"].astype(np.float64)[0:16]
    sin = inp["sint"].astype(np.float64)[0:16]
    out = np.zeros((S, E))
    for hh in range(2):
        wq = inp["wqc"].astype(np.float64)[0:64, 32 * hh : 32 * hh + 16]
        wqs = inp["wqc"].astype(np.float64)[64:128, 32 * hh : 32 * hh + 16]
        wk = inp["wkc"].astype(np.float64)[0:64, 32 * hh : 32 * hh + 16]
        wks = inp["wkc"].astype(np.float64)[64:128, 32 * hh : 32 * hh + 16]
        wv = inp["wv"].astype(np.float64)[:, 16 * hh : 16 * hh + 16]
        wo = inp["wo"].astype(np.float64)[:, hh, :]  # [16, 64]
        q = (x @ wq) * cos.T + (x @ wqs) * sin.T     # [S,16]
        k = (x @ wk) * cos.T + (x @ wks) * sin.T
        v = x @ wv
        s = q @ k.T
        mask = np.tril(np.ones((S, S), dtype=bool))
        p = np.where(mask, np.exp(s), 0.0)
        a = (p @ v) / p.sum(-1, keepdims=True)       # [S,16]
        out += a @ wo
    return out.astype(np.float32)


def build_nc(probe=None, amp=1, split_waits=True, ablate=None):
    """Build the (single, SPMD) Bass program.

    Pipeline: one fused loop over ci = 0..7.  Iteration ci projects+ropes
    K^T/Q^T chunk ci, builds V k-tiles 4ci..4ci+3, then runs causal
    attention for query chunk qc=ci (which only needs K/V up to k-tile
    4ci+3).  Projection work for ci+1 gap-fills PE stalls during attention
    of qc=ci.
    """
    import os
    ablate = ablate or os.environ.get("KABLATE") or ()
    import concourse.bass as bass
    import concourse.mybir as mybir
    import concourse.tile as tile

    f32 = mybir.dt.float32
    bf16 = mybir.dt.bfloat16
    i16 = mybir.dt.int16
    AF = mybir.ActivationFunctionType
    OP = mybir.AluOpType

    nc = bass.Bass()
    xt_d = nc.declare_dram_parameter("xt", [E, S], bf16, isOutput=False)
    wqc_d = nc.declare_dram_parameter("wqc", [128, 112], bf16, isOutput=False)
    wkc_d = nc.declare_dram_parameter("wkc", [128, 112], bf16, isOutput=False)
    wv_d = nc.declare_dram_parameter("wv", [E, 32], bf16, isOutput=False)
    wo_d = nc.declare_dram_parameter("wo", [D, 2, E], bf16, isOutput=False)
    cos_d = nc.declare_dram_parameter("cost", [112, S], bf16, isOutput=False)
    sin_d = nc.declare_dram_parameter("sint", [112, S], bf16, isOutput=False)
    idt_d = nc.declare_dram_parameter("idt", [128, 128], bf16, isOutput=False)
    out_d = nc.declare_dram_parameter("out", [S, E], f32, isOutput=True)

    with tile.TileContext(nc) as tc:
        with tc.tile_pool(name="persist", bufs=1) as pp:
            # ---- constants into SBUF ----
            # weight DMAs dispatch on the GpSimd queue, bulk tensors on the
            # Sync queue: each dma_start costs ~600ns of serial dispatch on
            # its issuing engine, so two queues halve the prologue
            wqc_sb = pp.tile([128, 112], bf16, name="wqc_sb")
            wkc_sb = pp.tile([128, 112], bf16, name="wkc_sb")
            wv_sb = pp.tile([E, 32], bf16, name="wv_sb")
            cos_sb = pp.tile([112, S], bf16, name="cos_sb")
            sin_sb = pp.tile([112, S], bf16, name="sin_sb")
            idt_sb = pp.tile([128, 128], bf16, name="idt_sb")
            # x^T duplicated onto partitions 64:128 so the shuffled-weight
            # projection can stream on PE row tile (64,0) concurrently
            xT = pp.tile([128, S], bf16, name="xT")
            wo_sb = pp.tile([128, E], bf16, name="wo_sb")
            for half in range(2):
                nc.sync.dma_start(xT[64 * half : 64 * half + E, 0:512],
                                  xt_d[:, 0:512])
            nc.sync.dma_start(wkc_sb, wkc_d[:])
            nc.sync.dma_start(wqc_sb, wqc_d[:])
            nc.sync.dma_start(wv_sb, wv_d[:])
            nc.sync.dma_start(cos_sb[:, 0:512], cos_d[:, 0:512])
            nc.sync.dma_start(sin_sb[:, 0:512], sin_d[:, 0:512])
            for hh in range(2):
                nc.sync.dma_start(
                    wo_sb[64 * hh : 64 * hh + D, :], wo_d[:, hh, :]
                )
            nc.sync.dma_start(idt_sb, idt_d[:])
            for half in range(2):
                nc.sync.dma_start(xT[64 * half : 64 * half + E, 512:S],
                                  xt_d[:, 512:S])
            for sb, dr in [(cos_sb, cos_d), (sin_sb, sin_d)]:
                nc.sync.dma_start(sb[:, 512:S], dr[:, 512:S])

            # ---- persistent activations ----
            rotK = pp.tile([112, S], bf16, name="rotK")
            rotQ = pp.tile([112, S], bf16, name="rotQ")
            vp = pp.tile([128, NKT, 2, 33], bf16, name="vp")
            attS_all = pp.tile([128, NQC, 512], bf16, name="attS_all")
            den2 = pp.tile([128, NQC, 4, 2], bf16, name="den2")
            nc.vector.memset(vp, 0.0)
            nc.vector.memset(vp[:, :, :, 32:33], 1.0)

            with tc.tile_pool(name="a_ps", bufs=1, space="PSUM") as sp, \
                 tc.tile_pool(name="a_pr", bufs=1, space="PSUM") as pr, \
                 tc.tile_pool(name="a_att", bufs=2, space="PSUM") as ap_, \
                 tc.tile_pool(name="a_ep", bufs=1, space="PSUM") as ep_, \
                 tc.tile_pool(name="a_p", bufs=8) as pb_, \
                 tc.tile_pool(name="a_n", bufs=2) as nb, \
                 tc.tile_pool(name="a_f", bufs=2) as fb:
              suid = [0]
              expc = [0]  # exp engine round-robin counter

              def stile(shape, dt, name):
                  # {projections, V, scores} time-share the four `sp` banks
                  # via a rotating 2-tag cycle (2 banks per tag)
                  t = sp.tile(shape, dt, tag=f"s{suid[0] % 2}", name=name)
                  suid[0] += 1
                  return t

              def emit_projw(ci, which):
                  # one projection + rope for chunk ci (emitted two chunks
                  # ahead of use, mid-attention; K and Q parts land on
                  # DIFFERENT pair-steps so the PE lump between score pairs
                  # stays small and the exp engines keep their backlog)
                  cs = slice(512 * ci, 512 * (ci + 1))
                  for wmat, rot, nm in ((wkc_sb, rotK, "k"),
                                        (wqc_sb, rotQ, "q"))[which:which + 1]:
                      # plain (rows 0:64, tile (0,0)) and rope-shuffled
                      # (rows 64:128, tile (64,0)) projections stream
                      # concurrently on the two 64-row PE tiles
                      pa = pr.tile([112, 512], f32, tag="pa", name=f"pa{nm}")
                      nc.tensor.matmul(pa, wmat[0:64, :], xT[0:64, cs],
                                       start=True, stop=True)
                      pb = pr.tile([112, 512], f32, tag="pa", name=f"pb{nm}")
                      nc.tensor.matmul(pb, wmat[64:128, :], xT[64:128, cs],
                                       start=True, stop=True)
                      t1 = nb.tile([112, 512], bf16, tag=f"t1{nm}",
                                   name=f"t1{nm}")
                      nc.vector.tensor_tensor(t1, pa, cos_sb[:, cs], OP.mult)
                      t2 = nb.tile([112, 512], bf16, tag=f"t2{nm}",
                                   name=f"t2{nm}")
                      nc.vector.tensor_tensor(t2, pb, sin_sb[:, cs], OP.mult)
                      nc.vector.tensor_tensor(rot[:, cs], t1, t2, OP.add)

              def emit_projv(ci):
                  # V projection: 4 k-tiles into one PSUM tile, one copy out
                  pv = ep_.tile([128, 4, 32], f32, tag="e0", name="pv")
                  for ii in range(4):
                      i = 4 * ci + ii
                      nc.tensor.matmul(
                          pv[:, ii, :], xT[0:E, 128 * i : 128 * (i + 1)],
                          wv_sb, start=True, stop=True,
                      )
                  nc.vector.tensor_copy(
                      vp[:, 4 * ci : 4 * ci + 4, :, 0:D],
                      pv.rearrange("p i (h d) -> p i h d", h=2),
                  )

              tail_state = {}

              def emit_tail(qc, half):
                  # normalize + output projection for chunk qc, row-major.
                  # Emitted DURING attention of qc+2, split across two
                  # pair-steps: its small PE/DVE ops fill engine slack
                  # instead of stalling chunk boundaries.
                  qs = slice(512 * qc, 512 * (qc + 1))
                  if half == 0:
                      rec = nb.tile([128, 4, 2], f32, tag="rec", name="rec")
                      nc.vector.reciprocal(rec, den2[:, qc, :, :])
                      out_sb = fb.tile([128, 4, E], f32, tag="ob", name="ob")
                      tail_state[qc] = (rec, out_sb)
                  else:
                      rec, out_sb = tail_state.pop(qc)
                  for c in (0, 1) if half == 0 else (2, 3):
                      # the two heads' (parity-accumulated) projections drain
                      # into DIFFERENT banks; both banks are borrowed from
                      # pools whose previous readers finished long ago
                      ops0 = ep_.tile([128, E], f32, tag="e0", name="ops0")
                      ops1 = pr.tile([128, E], f32, tag="pa", name="ops1")
                      for hh, opst in ((0, ops0), (1, ops1)):
                          base = 64 * hh
                          nc.tensor.matmul(
                              opst,
                              attS_all[base : base + D, qc,
                                       128 * c : 128 * (c + 1)],
                              wo_sb[base : base + D, :],
                              start=True, stop=True,
                              tile_position=(base, 0),
                          )
                      u0 = nb.tile([128, E], f32, tag="u0", name="u0")
                      nc.vector.tensor_scalar(
                          u0, ops0, rec[:, c, 0:1], None, OP.mult,
                      )
                      nc.vector.scalar_tensor_tensor(
                          out_sb[:, c, :], ops1, rec[:, c, 1:2], u0,
                          OP.mult, OP.add,
                      )
                  if half == 1:
                      nc.sync.dma_start(
                          out_d[qs, :].rearrange("(c p) e -> p c e", p=128),
                          out_sb,
                      )

              for ci in (0, 1):
                  emit_projw(ci, 0)
                  emit_projw(ci, 1)
                  emit_projv(ci)
              for qc in range(NQC):
                    nk = 4 * qc + 4
                    att = ap_.tile([128, 512], f32, tag="att", name="att")

                    def emit_att(kt_, pt_, q0_):
                        # two column tiles (head -> psum half); TRN2's 4x
                        # column tiling is unusable (XBUS quadrant-3 bug),
                        # so kt's serialize per head.  start=True on kt 0
                        # replaces the PSUM memset; row 64hh+32 accumulates
                        # the softmax denominator via vp's ones column
                        for hh in range(2):
                            nc.tensor.matmul(
                                att[64 * hh : 64 * hh + 33, q0_:512],
                                vp[:, kt_, hh, :], pt_[:, hh, q0_:512],
                                start=(kt_ == 0), stop=(kt_ == nk - 1),
                                skip_group_check=True,
                                tile_position=(0, 64 * hh),
                            )

                    pend: list = []
                    for g0 in range(0, nk, 2):
                        # kt pair: the FOUR score matmuls are adjacent in the
                        # PE queue and land in the four 32-row PE groups
                        # (kt parity -> row offset 0/64, head -> +0/+32) and
                        # four distinct PSUM banks.
                        if len(pend) >= 4:
                            for args in pend[:4]:
                                emit_att(*args)
                            del pend[:4]
                        pss = []
                        for kt in (g0, g0 + 1):
                            roff = 64 * (kt % 2)
                            r = 128 * kt - 512 * qc
                            q0 = max(r, 0)  # causal live columns [q0, 512)
                            ps = stile([128, 2, 512], f32, name="ps")
                            for hh in range(2):
                                base = roff + 32 * hh
                                nc.tensor.matmul(
                                    ps[:, hh, q0:512],
                                    rotK[base : base + D,
                                         128 * kt : 128 * (kt + 1)],
                                    rotQ[base : base + D,
                                         512 * qc + q0 : 512 * (qc + 1)],
                                    start=True, stop=True,
                                    tile_position=(base, 0),
                                )
                            pss.append((kt, ps, q0, r))
                        cur2 = []
                        for kt, ps, q0, r in pss:
                            pt = pb_.tile([128, 2, 512], bf16, tag="pt",
                                          name="pt")
                            if expc[0] % EXP_MOD == EXP_PHASE:
                                # Schraudolph exp2 on the DVE: one fused
                                # mult+add whose int16 output bits are the
                                # bf16 pattern of exp(s)
                                nc.vector.tensor_scalar(
                                    pt.bitcast(i16)[:, :, q0:512],
                                    ps[:, :, q0:512],
                                    EXPA, EXPB, OP.mult, OP.add,
                                )
                            else:
                                nc.scalar.activation(
                                    pt[:, :, q0:512], ps[:, :, q0:512],
                                    AF.Exp,
                                )
                            expc[0] += 1
                            if r >= 0 and "noaffine" not in ablate:
                                for hh in range(2):
                                    nc.gpsimd.affine_select(
                                        out=pt[:, hh, q0:512],
                                        in_=pt[:, hh, q0:512],
                                        pattern=[[1, 512 - q0]],
                                        compare_op=mybir.AluOpType.is_ge,
                                        fill=0.0, base=0,
                                        channel_multiplier=-1,
                                    )
                            cur2.append((kt, pt, q0))
                        # attended runs a step late and TWO pairs at a
                        # time (flushed at the TOP of the step, before the
                        # scores): half the row<->column PE mode switches,
                        # and the in-order PE queue meets the ps-tag wait
                        # only after the ready attended work
                        pend += cur2
                        if qc + 2 < NQC:
                            if g0 == 0:
                                emit_projw(qc + 2, 0)
                            elif g0 == 2:
                                emit_projw(qc + 2, 1)
                                if qc == 0:  # qc 0 has only two pair-steps
                                    emit_projv(qc + 2)
                            elif g0 == 4:
                                emit_projv(qc + 2)
                        if qc >= 2:
                            if g0 == 2:
                                emit_tail(qc - 2, 0)
                            elif g0 == 6:
                                emit_tail(qc - 2, 1)
                    for args in pend:
                        emit_att(*args)
                    # park the (unnormalized) attended chunk for the tail
                    nc.vector.tensor_copy(attS_all[:, qc, :], att)
                    # denominators (rows 32t+16) PE-transposed onto the
                    # q-partitions; the parity pair-sum lands in den2 for
                    # the deferred tail (consumed two chunks later)
                    attT = ep_.tile([128, 4, 128], bf16, tag="e0",
                                    name="attT")
                    for c in range(4):
                        nc.tensor.matmul(
                            attT[:, c, :],
                            attS_all[:, qc, 128 * c : 128 * (c + 1)],
                            idt_sb, is_transpose=True, start=True, stop=True,
                        )
                    nc.vector.tensor_copy(
                        den2[:, qc, :, :], attT[:, :, 32:97:64]
                    )
              for qc in (NQC - 2, NQC - 1):
                  emit_tail(qc, 0)
                  emit_tail(qc, 1)
    # populate .instr bytes for extended-inst InstISA subclasses (raw Bass
    # does not run this pass; without it walrus fails "ISA wrong length")
    from concourse.library_overlay import lower_extended_insts
    lower_extended_insts(nc)
    if split_waits:  # required for walrus; breaks CoreSim's race detector
        _split_multi_waits(nc, mybir)
    return nc


def _split_multi_waits(nc, mybir):
    """This walrus build accepts at most ONE sync-wait command per
    instruction ("Too many sync wait commands").  Tile emits instructions
    with several waits; hoist all but the last into standalone
    InstEventSemaphore (sequencer wait) instructions on the same engine,
    inserted immediately before."""
    import bass_rust

    uid = [0]
    for f in nc.m.functions:
        for blk in f.blocks:
            insts = list(blk.instructions)
            out = []
            changed = False
            for inst in insts:
                si = inst.sync_info
                waits = list(si.on_wait) if si is not None else []
                if len(waits) > 1:
                    changed = True
                    for w in waits[:-1]:
                        ev = mybir.InstEventSemaphore(
                            name=f"WSPLIT-{uid[0]}", ins=[], outs=[]
                        )
                        uid[0] += 1
                        ev.engine = inst.engine
                        ev.sync_info = bass_rust.SyncInfo(
                            on_wait=[w], on_update=[]
                        )
                        out.append(ev)
                    inst.sync_info = bass_rust.SyncInfo(
                        on_wait=[waits[-1]], on_update=list(si.on_update)
                    )
                out.append(inst)
            if changed:
                blk.instructions = out
    return nc


def _get_nc(probe=None):
    key = ("nc", probe)
    if key not in _CACHE:
        _CACHE[key] = build_nc(probe)
    return _CACHE[key]


def kernel(x, Wq, Wk, Wv, Wo):
    from concourse.bass_utils import run_bass_kernel_spmd

    x = np.asarray(x, dtype=np.float32)
    Wq, Wk, Wv, Wo = (np.asarray(w, dtype=np.float32) for w in (Wq, Wk, Wv, Wo))

    nc = _get_nc()
    in_maps = [make_core_inputs(x, Wq, Wk, Wv, Wo, c) for c in range(NCORES)]
    res = run_bass_kernel_spmd(nc, in_maps, core_ids=list(range(NCORES)))
    out = np.empty((B, S, E), dtype=np.float32)
    for b in range(B):
        out[b] = res.results[2 * b]["out"] + res.results[2 * b + 1]["out"]
    return out


# revision 16
# speedup vs baseline: 1.0573x; 1.0573x over previous
"""Causal self-attention with RoPE on 8 Trainium2 NeuronCores.

Problem: B=4, S=4096, E=64, H=4 heads x D=16, fp32 in/out.

Sharding: core c handles batch b = c//2 and head-pair hp = c%2 (heads 2*hp,
2*hp+1).  Every core runs the IDENTICAL program (SPMD) -- per-core behavior
comes only from the data (x[b] and per-head weight slices).  Each core
returns the partial output projection sum over its two heads; the host adds
the two partials per batch.

Device algorithm (per core, per head):
  - x^T [64,S] DMA'd pre-transposed (host does the transpose)
  - K^T,Q^T projections as lhsT.T@x^T (scale 1/sqrt(D) folded into Wq);
    RoPE applied as  rot = proj * cos + proj_shuf * sin  where proj_shuf
    comes from a sign/permuted weight matrix (R@W) -- no cross-partition ops
  - scores computed TRANSPOSED: S^T[k',q] over d=16 contraction, with the
    4 (kt-parity x head) score matmuls on the four 32-row PE row tiles
  - softmax exp: ScalarE ACTIVATE per kt-tile (a VectorE Schraudolph exp2
    fallback -- tensor_scalar mult+add writing int16 bits that ARE the
    bf16 pattern of 2^(s*log2e) -- is selectable via KEXP_MOD but measured
    slower: the DVE is the secondary pacer)
  - causal mask applied post-exp with affine_select (fill 0) on GpSimd
  - attended^T accumulated on the two 64-partition PE column tiles (head
    -> psum half), V augmented with a ones column so row 64hh+32
    accumulates the softmax denominator; attended trails the scores by two
    kt-pairs so row<->column PE mode switches halve
  - normalize with reciprocal + per-chunk scalar multiplies, PE-transposed
    denominators, output DMA'd row-major; projection/normalize/V work is
    spread across kt-pair steps so the exp engine never starves
"""

import sys

sys.path.insert(0, "/opt/trn_rl_repo")

import numpy as np
import ml_dtypes

B, S, E, H, D = 4, 4096, 64, 4, 16
NCORES = 8
NKT = S // 128  # 32 k-tiles of 128
NQC = S // 512  # 8 q-chunks of 512

BF16 = ml_dtypes.bfloat16

# Schraudolph fast-exp on bf16 bits: bits = rint(s*EXPA + EXPB); the int16
# pattern read as bf16 is exp(s)*(1 +- 3.3%), zero-ish mean after softmax
# normalization.  Validated bit-exact vs numpy rint on HW.
LOG2E = 1.4426950408889634
EXPA = 128.0 * LOG2E
EXPC = 5.60
EXPB = 127.0 * 128.0 - EXPC
# fraction of kt-tiles whose exp runs on the DVE instead of ScalarE:
# tile i -> DVE iff i % EXP_MOD == EXP_PHASE.  Default: ALL tiles on
# ScalarE -- measured faster than any DVE offload split, because the DVE
# (rope multiplies + casts + normalize tail) is the secondary pacer and
# exp tiles in its queue delay the score->attended chain, while ScalarE
# has slack.  The Schraudolph path stays available via KEXP_MOD.
EXP_MOD = int(__import__('os').environ.get('KEXP_MOD', 1 << 30))
EXP_PHASE = 2

_CACHE: dict = {}


def _rope_tables():
    # cos/sin[16*hh + d, s] = cos/sin(s * invfreq[d//2]); same for both heads
    pos = np.arange(S, dtype=np.float64)
    pair = np.arange(0, D, 2, dtype=np.float64)  # 0,2,..,14
    inv = 1.0 / (10000.0 ** (pair / D))          # [8]
    ang = pos[None, :] * inv[:, None]            # [8, S]
    cos8, sin8 = np.cos(ang), np.sin(ang)
    cos16 = np.repeat(cos8, 2, axis=0)           # [16, S] rows 2p,2p+1 equal
    sin16 = np.repeat(sin8, 2, axis=0)
    z16 = np.zeros_like(cos16)
    # [112, S]: rows 0-15, 32-47, 64-79, 96-111 hold cos16 (both heads, both
    # kt-parity copies); gap rows zero.
    cos112 = np.concatenate([cos16, z16, cos16, z16, cos16, z16, cos16], axis=0)
    sin112 = np.concatenate([sin16, z16, sin16, z16, sin16, z16, sin16], axis=0)
    return cos112.astype(BF16), sin112.astype(BF16)


def _shuffle_rows(w):
    # (R w)[2p] = -w[2p+1], (R w)[2p+1] = w[2p]   (rope partner)
    ws = np.empty_like(w)
    ws[0::2] = -w[1::2]
    ws[1::2] = w[0::2]
    return ws


def make_core_inputs(x, Wq, Wk, Wv, Wo, core):
    """Build the per-core input map (all host-side numpy)."""
    b, hp = core // 2, core % 2
    rs = slice(32 * hp, 32 * hp + 32)  # rows of the 2 heads in W{q,k,v}
    scale = 1.0 / np.sqrt(np.float32(D))

    wq_sel = (Wq[rs] * scale).astype(np.float32)  # [32, 64]
    wk_sel = Wk[rs].astype(np.float32)
    cos112, sin112 = _CACHE.setdefault("rope", _rope_tables())

    def gap112(w32):
        # [32,64] head rows -> [64,112] lhsT with head hh at cols
        # {32*hh, 64+32*hh} + 0:16 (duplicated for the kt-parity row groups)
        out = np.zeros((64, 112), np.float32)
        out[:, 0:16] = w32[0:16].T
        out[:, 32:48] = w32[16:32].T
        out[:, 64:80] = w32[0:16].T
        out[:, 96:112] = w32[16:32].T
        return out

    # plain and rope-shuffled projection weights stacked [128, 112]: rows
    # 0:64 feed PE row tile (0,0), rows 64:128 feed tile (64,0) so the two
    # projection matmuls stream concurrently
    wqc = np.concatenate([gap112(wq_sel), gap112(_shuffle_rows(wq_sel))], 0)
    wkc = np.concatenate([gap112(wk_sel), gap112(_shuffle_rows(wk_sel))], 0)
    return {
        "xt": np.ascontiguousarray(x[b].T).astype(BF16),              # [64,S]
        "wqc": np.ascontiguousarray(wqc).astype(BF16),                # [128,112]
        "wkc": np.ascontiguousarray(wkc).astype(BF16),
        "wv": np.ascontiguousarray(Wv[rs].T).astype(BF16),            # [64,32]
        # wo[d, hh, e] = Wo[e, 16*(2hp+hh)+d]
        "wo": np.ascontiguousarray(
            Wo[:, rs].reshape(E, 2, D).transpose(2, 1, 0)
        ).astype(BF16),                                               # [16,2,64]
        "cost": cos112,
        "sint": sin112,
        "idt": np.eye(128, dtype=BF16),
    }


def partial_reference(inp):
    """Numpy reference of ONE core's partial output (for testing)."""
    x = inp["xt"].astype(np.float64).T
    cos = inp["cost"].astype(np.float64)[0:16]
    sin = inp["sint"].astype(np.float64)[0:16]
    out = np.zeros((S, E))
    for hh in range(2):
        wq = inp["wqc"].astype(np.float64)[0:64, 32 * hh : 32 * hh + 16]
        wqs = inp["wqc"].astype(np.float64)[64:128, 32 * hh : 32 * hh + 16]
        wk = inp["wkc"].astype(np.float64)[0:64, 32 * hh : 32 * hh + 16]
        wks = inp["wkc"].astype(np.float64)[64:128, 32 * hh : 32 * hh + 16]
        wv = inp["wv"].astype(np.float64)[:, 16 * hh : 16 * hh + 16]
        wo = inp["wo"].astype(np.float64)[:, hh, :]  # [16, 64]
        q = (x @ wq) * cos.T + (x @ wqs) * sin.T     # [S,16]
        k = (x @ wk) * cos.T + (x @ wks) * sin.T
        v = x @ wv
        s = q @ k.T
        mask = np.tril(np.ones((S, S), dtype=bool))
        p = np.where(mask, np.exp(s), 0.0)
        a = (p @ v) / p.sum(-1, keepdims=True)       # [S,16]
        out += a @ wo
    return out.astype(np.float32)


def build_nc(probe=None, amp=1, split_waits=True, ablate=None):
    """Build the (single, SPMD) Bass program.

    Pipeline: one fused loop over ci = 0..7.  Iteration ci projects+ropes
    K^T/Q^T chunk ci, builds V k-tiles 4ci..4ci+3, then runs causal
    attention for query chunk qc=ci (which only needs K/V up to k-tile
    4ci+3).  Projection work for ci+1 gap-fills PE stalls during attention
    of qc=ci.
    """
    import os
    ablate = ablate or os.environ.get("KABLATE") or ()
    import concourse.bass as bass
    import concourse.mybir as mybir
    import concourse.tile as tile

    f32 = mybir.dt.float32
    bf16 = mybir.dt.bfloat16
    i16 = mybir.dt.int16
    AF = mybir.ActivationFunctionType
    OP = mybir.AluOpType

    nc = bass.Bass()
    xt_d = nc.declare_dram_parameter("xt", [E, S], bf16, isOutput=False)
    wqc_d = nc.declare_dram_parameter("wqc", [128, 112], bf16, isOutput=False)
    wkc_d = nc.declare_dram_parameter("wkc", [128, 112], bf16, isOutput=False)
    wv_d = nc.declare_dram_parameter("wv", [E, 32], bf16, isOutput=False)
    wo_d = nc.declare_dram_parameter("wo", [D, 2, E], bf16, isOutput=False)
    cos_d = nc.declare_dram_parameter("cost", [112, S], bf16, isOutput=False)
    sin_d = nc.declare_dram_parameter("sint", [112, S], bf16, isOutput=False)
    idt_d = nc.declare_dram_parameter("idt", [128, 128], bf16, isOutput=False)
    out_d = nc.declare_dram_parameter("out", [S, E], f32, isOutput=True)

    with tile.TileContext(nc) as tc:
        with tc.tile_pool(name="persist", bufs=1) as pp:
            # ---- constants into SBUF ----
            # weight DMAs dispatch on the GpSimd queue, bulk tensors on the
            # Sync queue: each dma_start costs ~600ns of serial dispatch on
            # its issuing engine, so two queues halve the prologue
            wqc_sb = pp.tile([128, 112], bf16, name="wqc_sb")
            wkc_sb = pp.tile([128, 112], bf16, name="wkc_sb")
            wv_sb = pp.tile([E, 32], bf16, name="wv_sb")
            cos_sb = pp.tile([112, S], bf16, name="cos_sb")
            sin_sb = pp.tile([112, S], bf16, name="sin_sb")
            idt_sb = pp.tile([128, 128], bf16, name="idt_sb")
            # x^T duplicated onto partitions 64:128 so the shuffled-weight
            # projection can stream on PE row tile (64,0) concurrently
            xT = pp.tile([128, S], bf16, name="xT")
            wo_sb = pp.tile([128, E], bf16, name="wo_sb")
            for half in range(2):
                nc.sync.dma_start(xT[64 * half : 64 * half + E, 0:512],
                                  xt_d[:, 0:512])
            nc.sync.dma_start(wkc_sb, wkc_d[:])
            nc.sync.dma_start(wqc_sb, wqc_d[:])
            nc.sync.dma_start(wv_sb, wv_d[:])
            nc.sync.dma_start(cos_sb[:, 0:512], cos_d[:, 0:512])
            nc.sync.dma_start(sin_sb[:, 0:512], sin_d[:, 0:512])
            for hh in range(2):
                nc.sync.dma_start(
                    wo_sb[64 * hh : 64 * hh + D, :], wo_d[:, hh, :]
                )
            nc.sync.dma_start(idt_sb, idt_d[:])
            for half in range(2):
                nc.sync.dma_start(xT[64 * half : 64 * half + E, 512:S],
                                  xt_d[:, 512:S])
            for sb, dr in [(cos_sb, cos_d), (sin_sb, sin_d)]:
                nc.sync.dma_start(sb[:, 512:S], dr[:, 512:S])

            # ---- persistent activations ----
            rotK = pp.tile([112, S], bf16, name="rotK")
            rotQ = pp.tile([112, S], bf16, name="rotQ")
            vp = pp.tile([128, NKT, 2, 33], bf16, name="vp")
            attS_all = pp.tile([128, NQC, 512], bf16, name="attS_all")
            den2 = pp.tile([128, NQC, 4, 2], bf16, name="den2")
            nc.vector.memset(vp, 0.0)
            nc.vector.memset(vp[:, :, :, 32:33], 1.0)

            with tc.tile_pool(name="a_ps", bufs=1, space="PSUM") as sp, \
                 tc.tile_pool(name="a_pr", bufs=1, space="PSUM") as pr, \
                 tc.tile_pool(name="a_att", bufs=1, space="PSUM") as ap_, \
                 tc.tile_pool(name="a_ep", bufs=1, space="PSUM") as ep_, \
                 tc.tile_pool(name="a_p", bufs=8) as pb_, \
                 tc.tile_pool(name="a_n", bufs=2) as nb, \
                 tc.tile_pool(name="a_f", bufs=2) as fb:
              suid = [0]
              expc = [0]  # exp engine round-robin counter

              def stile(shape, dt, name):
                  # {projections, V, scores} time-share the four `sp` banks
                  # via a rotating 2-tag cycle (2 banks per tag)
                  t = sp.tile(shape, dt, tag=f"s{suid[0] % 2}", name=name)
                  suid[0] += 1
                  return t

              def emit_projw(ci, which):
                  # one projection + rope for chunk ci (emitted two chunks
                  # ahead of use, mid-attention; K and Q parts land on
                  # DIFFERENT pair-steps so the PE lump between score pairs
                  # stays small and the exp engines keep their backlog)
                  cs = slice(512 * ci, 512 * (ci + 1))
                  for wmat, rot, nm in ((wkc_sb, rotK, "k"),
                                        (wqc_sb, rotQ, "q"))[which:which + 1]:
                      # plain (rows 0:64, tile (0,0)) and rope-shuffled
                      # (rows 64:128, tile (64,0)) projections stream
                      # concurrently on the two 64-row PE tiles
                      pa = pr.tile([112, 512], f32, tag="pa", name=f"pa{nm}")
                      nc.tensor.matmul(pa, wmat[0:64, :], xT[0:64, cs],
                                       start=True, stop=True)
                      pb = pr.tile([112, 512], f32, tag="pb", name=f"pb{nm}")
                      nc.tensor.matmul(pb, wmat[64:128, :], xT[64:128, cs],
                                       start=True, stop=True)
                      t1 = nb.tile([112, 512], bf16, tag=f"t1{nm}",
                                   name=f"t1{nm}")
                      nc.vector.tensor_tensor(t1, pa, cos_sb[:, cs], OP.mult)
                      t2 = nb.tile([112, 512], bf16, tag=f"t2{nm}",
                                   name=f"t2{nm}")
                      nc.vector.tensor_tensor(t2, pb, sin_sb[:, cs], OP.mult)
                      nc.vector.tensor_tensor(rot[:, cs], t1, t2, OP.add)

              def emit_projv(ci):
                  # V projection: 4 k-tiles into one PSUM tile, one copy out
                  pv = ep_.tile([128, 4, 32], f32, tag="e0", name="pv")
                  for ii in range(4):
                      i = 4 * ci + ii
                      nc.tensor.matmul(
                          pv[:, ii, :], xT[0:E, 128 * i : 128 * (i + 1)],
                          wv_sb, start=True, stop=True,
                      )
                  nc.vector.tensor_copy(
                      vp[:, 4 * ci : 4 * ci + 4, :, 0:D],
                      pv.rearrange("p i (h d) -> p i h d", h=2),
                  )

              tail_state = {}

              def emit_tail(qc, half):
                  # normalize + output projection for chunk qc, row-major.
                  # Emitted DURING attention of qc+2, split across two
                  # pair-steps: its small PE/DVE ops fill engine slack
                  # instead of stalling chunk boundaries.
                  qs = slice(512 * qc, 512 * (qc + 1))
                  if half == 0:
                      rec = nb.tile([128, 4, 2], f32, tag="rec", name="rec")
                      nc.vector.reciprocal(rec, den2[:, qc, :, :])
                      out_sb = fb.tile([128, 4, E], f32, tag="ob", name="ob")
                      tail_state[qc] = (rec, out_sb)
                  else:
                      rec, out_sb = tail_state.pop(qc)
                  for c in (0, 1) if half == 0 else (2, 3):
                      # the two heads' (parity-accumulated) projections drain
                      # into DIFFERENT banks; both banks are borrowed from
                      # pools whose previous readers finished long ago
                      ops0 = ep_.tile([128, E], f32, tag="e0", name="ops0")
                      ops1 = pr.tile([128, E], f32,
                                     tag=("pa", "pb")[c % 2], name="ops1")
                      for hh, opst in ((0, ops0), (1, ops1)):
                          base = 64 * hh
                          nc.tensor.matmul(
                              opst,
                              attS_all[base : base + D, qc,
                                       128 * c : 128 * (c + 1)],
                              wo_sb[base : base + D, :],
                              start=True, stop=True,
                              tile_position=(base, 0),
                          )
                      u0 = nb.tile([128, E], f32, tag="u0", name="u0")
                      nc.vector.tensor_scalar(
                          u0, ops0, rec[:, c, 0:1], None, OP.mult,
                      )
                      nc.vector.scalar_tensor_tensor(
                          out_sb[:, c, :], ops1, rec[:, c, 1:2], u0,
                          OP.mult, OP.add,
                      )
                  if half == 1:
                      nc.sync.dma_start(
                          out_d[qs, :].rearrange("(c p) e -> p c e", p=128),
                          out_sb,
                      )

              for ci in (0, 1):
                  emit_projw(ci, 0)
                  emit_projw(ci, 1)
                  emit_projv(ci)
              for qc in range(NQC):
                    nk = 4 * qc + 4
                    att = ap_.tile([128, 512], f32, tag="att", name="att")

                    def emit_att(kt_, pt_, q0_):
                        # two column tiles (head -> psum half); TRN2's 4x
                        # column tiling is unusable (XBUS quadrant-3 bug),
                        # so kt's serialize per head.  start=True on kt 0
                        # replaces the PSUM memset; row 64hh+32 accumulates
                        # the softmax denominator via vp's ones column
                        for hh in range(2):
                            nc.tensor.matmul(
                                att[64 * hh : 64 * hh + 33, q0_:512],
                                vp[:, kt_, hh, :], pt_[:, hh, q0_:512],
                                start=(kt_ == 0), stop=(kt_ == nk - 1),
                                skip_group_check=True,
                                tile_position=(0, 64 * hh),
                            )

                    pend: list = []
                    for g0 in range(0, nk, 2):
                        # kt pair: the FOUR score matmuls are adjacent in the
                        # PE queue and land in the four 32-row PE groups
                        # (kt parity -> row offset 0/64, head -> +0/+32) and
                        # four distinct PSUM banks.
                        pss = []
                        for kt in (g0, g0 + 1):
                            roff = 64 * (kt % 2)
                            r = 128 * kt - 512 * qc
                            q0 = max(r, 0)  # causal live columns [q0, 512)
                            ps = stile([128, 2, 512], f32, name="ps")
                            for hh in range(2):
                                base = roff + 32 * hh
                                nc.tensor.matmul(
                                    ps[:, hh, q0:512],
                                    rotK[base : base + D,
                                         128 * kt : 128 * (kt + 1)],
                                    rotQ[base : base + D,
                                         512 * qc + q0 : 512 * (qc + 1)],
                                    start=True, stop=True,
                                    tile_position=(base, 0),
                                )
                            pss.append((kt, ps, q0, r))
                        cur2 = []
                        for kt, ps, q0, r in pss:
                            pt = pb_.tile([128, 2, 512], bf16, tag="pt",
                                          name="pt")
                            if expc[0] % EXP_MOD == EXP_PHASE:
                                # Schraudolph exp2 on the DVE: one fused
                                # mult+add whose int16 output bits are the
                                # bf16 pattern of exp(s)
                                nc.vector.tensor_scalar(
                                    pt.bitcast(i16)[:, :, q0:512],
                                    ps[:, :, q0:512],
                                    EXPA, EXPB, OP.mult, OP.add,
                                )
                            else:
                                nc.scalar.activation(
                                    pt[:, :, q0:512], ps[:, :, q0:512],
                                    AF.Exp,
                                )
                            expc[0] += 1
                            if r >= 0 and "noaffine" not in ablate:
                                for hh in range(2):
                                    nc.gpsimd.affine_select(
                                        out=pt[:, hh, q0:512],
                                        in_=pt[:, hh, q0:512],
                                        pattern=[[1, 512 - q0]],
                                        compare_op=mybir.AluOpType.is_ge,
                                        fill=0.0, base=0,
                                        channel_multiplier=-1,
                                    )
                            cur2.append((kt, pt, q0))
                        # attended runs a step late and TWO pairs at a
                        # time: half the row<->column PE mode switches, and
                        # pt lives in SBUF so the extra lag is free
                        if len(pend) >= 4:
                            for args in pend[:4]:
                                emit_att(*args)
                            del pend[:4]
                        pend += cur2
                        if qc + 2 < NQC:
                            if g0 == 0:
                                emit_projw(qc + 2, 0)
                            elif g0 == 2:
                                emit_projw(qc + 2, 1)
                                if qc == 0:  # qc 0 has only two pair-steps
                                    emit_projv(qc + 2)
                            elif g0 == 4:
                                emit_projv(qc + 2)
                        if qc >= 2:
                            if g0 == 2:
                                emit_tail(qc - 2, 0)
                            elif g0 == 6:
                                emit_tail(qc - 2, 1)
                    for args in pend:
                        emit_att(*args)
                    # park the (unnormalized) attended chunk for the tail
                    nc.vector.tensor_copy(attS_all[:, qc, :], att)
                    # denominators (rows 32t+16) PE-transposed onto the
                    # q-partitions; the parity pair-sum lands in den2 for
                    # the deferred tail (consumed two chunks later)
                    attT = ep_.tile([128, 4, 128], bf16, tag="e0",
                                    name="attT")
                    for c in range(4):
                        nc.tensor.matmul(
                            attT[:, c, :],
                            attS_all[:, qc, 128 * c : 128 * (c + 1)],
                            idt_sb, is_transpose=True, start=True, stop=True,
                        )
                    nc.vector.tensor_copy(
                        den2[:, qc, :, :], attT[:, :, 32:97:64]
                    )
              for qc in (NQC - 2, NQC - 1):
                  emit_tail(qc, 0)
                  emit_tail(qc, 1)
    # populate .instr bytes for extended-inst InstISA subclasses (raw Bass
    # does not run this pass; without it walrus fails "ISA wrong length")
    from concourse.library_overlay import lower_extended_insts
    lower_extended_insts(nc)
    if split_waits:  # required for walrus; breaks CoreSim's race detector
        _split_multi_waits(nc, mybir)
    return nc


def _split_multi_waits(nc, mybir):
    """This walrus build accepts at most ONE sync-wait command per
    instruction ("Too many sync wait commands").  Tile emits instructions
    with several waits; hoist all but the last into standalone
    InstEventSemaphore (sequencer wait) instructions on the same engine,
    inserted immediately before."""
    import bass_rust

    uid = [0]
    for f in nc.m.functions:
        for blk in f.blocks:
            insts = list(blk.instructions)
            out = []
            changed = False
            for inst in insts:
                si = inst.sync_info
                waits = list(si.on_wait) if si is not None else []
                if len(waits) > 1:
                    changed = True
                    for w in waits[:-1]:
                        ev = mybir.InstEventSemaphore(
                            name=f"WSPLIT-{uid[0]}", ins=[], outs=[]
                        )
                        uid[0] += 1
                        ev.engine = inst.engine
                        ev.sync_info = bass_rust.SyncInfo(
                            on_wait=[w], on_update=[]
                        )
                        out.append(ev)
                    inst.sync_info = bass_rust.SyncInfo(
                        on_wait=[waits[-1]], on_update=list(si.on_update)
                    )
                out.append(inst)
            if changed:
                blk.instructions = out
    return nc


def _get_nc(probe=None):
    key = ("nc", probe)
    if key not in _CACHE:
        _CACHE[key] = build_nc(probe)
    return _CACHE[key]


def kernel(x, Wq, Wk, Wv, Wo):
    from concourse.bass_utils import run_bass_kernel_spmd

    x = np.asarray(x, dtype=np.float32)
    Wq, Wk, Wv, Wo = (np.asarray(w, dtype=np.float32) for w in (Wq, Wk, Wv, Wo))

    nc = _get_nc()
    in_maps = [make_core_inputs(x, Wq, Wk, Wv, Wo, c) for c in range(NCORES)]
    res = run_bass_kernel_spmd(nc, in_maps, core_ids=list(range(NCORES)))
    out = np.empty((B, S, E), dtype=np.float32)
    for b in range(B):
        out[b] = res.results[2 * b]["out"] + res.results[2 * b + 1]["out"]
    return out
